# revision 2
# baseline (speedup 1.0000x reference)
"""Trainium2 Bass kernel for nn_Arch7V3GraphEncoder (gnn_message_passing), v2.

Same 8-way graph/data-parallel sharding as v1 (canonical nodes partitioned
across cores, host sums the per-core [32,128] partials), with a restructured
device kernel driven by cost-model profiling of v1:

  - v1 spent ~400us of SP-sequencer time on 349 DMAs (d/b one-hots re-streamed
    every layer) and ~300us of PE-sequencer time on 8K+ Ldweights/Matmult
    pairs, with ACT ~95% / DVE ~87% busy on many small PSUM->SBUF ops.
  - v2 keeps the s/d one-hots PERSISTENT in SBUF as fp8 (exact for 0/1
    values; mixed fp8-stationary x bf16-moving matmuls verified exact on hw),
    streams only the tiny bond one-hot, batches all elementwise ops to
    [128,512], balances them across ACT/DVE via knobs, and bulk-loads inputs
    through the idle GPSIMD SWDGE queue at startup.
  - The role embedding is folded into the atom table (atom2 = atom + role[1],
    plus a rank-1 (role[0]-role[1]) x rootmask matmul), removing the role
    one-hot entirely.
"""

import sys

sys.path.insert(0, "/opt/trn_rl_repo")

import numpy as np
import ml_dtypes

BF16 = ml_dtypes.bfloat16
F8 = ml_dtypes.float8_e4m3

# Problem constants (hardcoded per spec).
N_TOTAL = 4096
M_SUB = 4
K_NODES = 8
L_LAYERS = 4
H = 128
NUM_GRAPHS = 32
IN_CH = 119
EDGE_DIM = 8
S_ALL = N_TOTAL * M_SUB          # 16384 subgraphs
SK_ALL = S_ALL * K_NODES         # 131072 flat nodes
E_ALL = 12 * S_ALL               # 196608 edges
NCORES = 8
S_LOC = S_ALL // NCORES          # 2048 subgraphs / core
SK_LOC = SK_ALL // NCORES        # 16384 flat nodes / core
NT = SK_LOC // 128               # 128 tiles of 128 nodes
SG_T = 16                        # subgraphs per tile
NCAN_LOC = N_TOTAL // NCORES     # 512 canonical nodes / core
NQ = NCAN_LOC // 128             # 4 canonical chunks of 128
NG = NT // 4                     # 32 groups of 4 tiles (512 nodes)
E_CAP = 256                      # edge slots per tile (2 chunks of 128)

# ---- tuning knobs (engine routing / batching) ----
RELU_PAT = "A"       # engine per msg-relu pair (A=ACT, D=DVE), rotated
Y1_PAT = "A"
OUT_PAT = "A"
COPY_PAT = "D"       # transpose-copy engine rotation
# per-layer transpose routing for the 4 tiles of each group:
# P=PE+copy, S=SP-issued DMA transpose, A=ACT-issued DMA transpose
TRANS_SPEC = ["PPSS", "PPSS", "PPSS", "PPSS"]
EMBED_TRANS = "PPPP"  # same routing for the embed-phase transposes
POOL_OVL = False     # overlap subgraph-pool matmuls with layer 4
S_CHUNKS = 8         # startup chunks for s_oh / d_oh / b_oh
X_CHUNK = 512        # xoh DMA chunk columns
PSUM_M = 3           # psum bufs: msg
PSUM_Z = 2           # psum bufs: aggr
PSUM_Y = 2           # psum bufs: mlp
PSUM_T = 1           # psum bufs: transpose staging


def _host_preprocess(inputs):
    """Integer index preprocessing -> per-core one-hot / weight arrays."""
    x_tokens = np.asarray(inputs["x_tokens"]).astype(np.int64)
    edge_tokens = np.asarray(inputs["edge_tokens"]).astype(np.int64)
    intra_ei = np.asarray(inputs["intra_ei"]).astype(np.int64)
    node_ids = np.asarray(inputs["node_ids"]).astype(np.int64)
    valid = np.asarray(inputs["valid"]).astype(bool)
    log_probs = np.asarray(inputs["log_probs"]).astype(np.float32)
    batch_graph = np.asarray(inputs["batch_graph"]).astype(np.int64)

    src, dst = intra_ei[0], intra_ei[1]
    e_sub = src // K_NODES
    assert np.array_equal(dst // K_NODES, e_sub), "edges must be intra-subgraph"

    core_of_e = e_sub // S_LOC
    tile_of_e = (e_sub % S_LOC) // SG_T
    key = core_of_e * NT + tile_of_e
    counts = np.bincount(key, minlength=NCORES * NT)
    assert counts.max() <= E_CAP, f"edge overflow: {counts.max()} > {E_CAP}"

    order = np.argsort(key, kind="stable")
    starts = np.zeros(NCORES * NT, dtype=np.int64)
    starts[1:] = np.cumsum(counts)[:-1]
    slot = np.empty(E_ALL, dtype=np.int64)
    slot[order] = np.arange(E_ALL) - starts[key[order]]

    ec = NT * E_CAP
    src_loc = (src % 128).astype(np.int64)
    dst_loc = (dst % 128).astype(np.int64)
    col = tile_of_e * E_CAP + slot                       # column in S/B layout
    chunk = slot // 128
    e_loc = slot % 128
    dcol = tile_of_e * E_CAP + chunk * 128 + dst_loc     # column in D layout

    S_oh = np.zeros((NCORES, 128, ec), dtype=F8)
    D_oh = np.zeros((NCORES, 128, ec), dtype=F8)
    B_oh = np.zeros((NCORES, 8, ec), dtype=F8)
    S_oh[core_of_e, src_loc, col] = valid[src].astype(F8)
    D_oh[core_of_e, e_loc, dcol] = np.asarray(1, dtype=F8)
    B_oh[core_of_e, edge_tokens, col] = np.asarray(1, dtype=F8)

    j = np.arange(SK_ALL)
    j_core = j // SK_LOC
    j_loc = j % SK_LOC
    Xoh = np.zeros((NCORES, 128, SK_LOC), dtype=F8)
    Xoh[j_core, x_tokens, j_loc] = np.asarray(1, dtype=F8)

    # Subgraph pooling one-hot (vm mask folded in; division handled later).
    vm = node_ids >= 0
    P1 = np.zeros((NCORES, 128, NT * SG_T), dtype=BF16)
    tile_of_j = j_loc // 128
    P1[j_core, j_loc % 128, tile_of_j * SG_T + (j_loc % 128) // K_NODES] = vm.astype(
        BF16
    )
    cnt = np.bincount(j // K_NODES, weights=vm.astype(np.float64), minlength=S_ALL)
    recip_cnt = (1.0 / np.maximum(cnt, 1.0)).astype(np.float32).reshape(NCORES, 1, S_LOC)

    n = np.arange(N_TOTAL)
    Gmat = np.zeros((NCORES, 128, NQ * NUM_GRAPHS), dtype=BF16)
    Gmat[n // NCAN_LOC, n % 128, ((n % NCAN_LOC) // 128) * NUM_GRAPHS + batch_graph] = (
        np.asarray(1, dtype=BF16)
    )

    lp = log_probs.reshape(NCORES, 1, S_LOC).astype(np.float32)

    atom_emb = np.asarray(inputs["atom_emb"]).astype(np.float32)
    role_emb = np.asarray(inputs["role_emb"]).astype(np.float32)
    # reference: role = role_emb[is_root] -> roots get row 1, others row 0
    atom2 = np.zeros((128, H), dtype=BF16)
    atom2[:IN_CH] = (atom_emb + role_emb[0]).astype(BF16)
    diff = (role_emb[1] - role_emb[0]).reshape(1, H).astype(BF16)
    rootmask = (np.arange(X_CHUNK) % K_NODES == 0).reshape(1, X_CHUNK).astype(BF16)

    w1 = np.asarray(inputs["mlp_w1"]).astype(BF16)   # [L,H,H]
    w2 = np.asarray(inputs["mlp_w2"]).astype(BF16)
    wpack = np.concatenate(
        [w1.transpose(1, 0, 2).reshape(H, L_LAYERS * H),
         w2.transpose(1, 0, 2).reshape(H, L_LAYERS * H)], axis=1
    )  # [128, 2*L*H]: w1_l at cols l*H, w2_l at cols (L+l)*H
    bpack = np.concatenate(
        [np.asarray(inputs["mlp_b1"]).astype(np.float32).T,
         np.asarray(inputs["mlp_b2"]).astype(np.float32).T], axis=1
    )  # [128, 2*L]

    per_core = []
    for c in range(NCORES):
        per_core.append(
            {
                "s_oh": np.ascontiguousarray(S_oh[c]),
                "d_oh": np.ascontiguousarray(D_oh[c]),
                "b_oh": np.ascontiguousarray(B_oh[c]),
                "xoh": np.ascontiguousarray(Xoh[c]),
                "p1": np.ascontiguousarray(P1[c]),
                "gmat": np.ascontiguousarray(Gmat[c]),
                "recip_cnt": np.ascontiguousarray(recip_cnt[c]),
                "lp": np.ascontiguousarray(lp[c]),
            }
        )

    shared = {
        "atom2": atom2,
        "diff": diff,
        "rootmask": rootmask,
        "bond_emb": np.asarray(inputs["bond_emb"]).astype(BF16),
        "wpack": np.ascontiguousarray(wpack),
        "bpack": np.ascontiguousarray(bpack),
        "eps": np.asarray(inputs["eps"]).astype(np.float32).reshape(1, L_LAYERS),
        "alpha": np.asarray(inputs["ht_alpha"]).astype(np.float32).reshape(1, 1),
        "ones128": np.ones((1, 128), dtype=np.float32),
        "ident": np.eye(128, dtype=BF16),
    }
    return per_core, shared


def _build_bass(repeat=1):
    import concourse.bass as bass
    import concourse.mybir as mybir
    from concourse import bacc
    from concourse.tile import TileContext

    f32 = mybir.dt.float32
    bf16 = mybir.dt.bfloat16
    fp8 = mybir.dt.float8e4
    AF = mybir.ActivationFunctionType
    ALU = mybir.AluOpType
    AX = mybir.AxisListType

    ec = NT * E_CAP

    nc = bacc.Bacc("TRN2", target_bir_lowering=False, debug=False, num_devices=NCORES)

    def din(name, shape, dt):
        return nc.dram_tensor(name, shape, dt, kind="ExternalInput").ap()

    s_d = din("s_oh", [128, ec], fp8)
    d_d = din("d_oh", [128, ec], fp8)
    b_d = din("b_oh", [8, ec], fp8)
    x_d = din("xoh", [128, SK_LOC], fp8)
    p1_d = din("p1", [128, NT * SG_T], bf16)
    g_d = din("gmat", [128, NQ * NUM_GRAPHS], bf16)
    rc_d = din("recip_cnt", [1, S_LOC], f32)
    lp_d = din("lp", [1, S_LOC], f32)
    atom_d = din("atom2", [128, H], bf16)
    diff_d = din("diff", [1, H], bf16)
    rm_d = din("rootmask", [1, X_CHUNK], bf16)
    bond_d = din("bond_emb", [8, H], bf16)
    wp_d = din("wpack", [128, 2 * L_LAYERS * H], bf16)
    bp_d = din("bpack", [128, 2 * L_LAYERS], f32)
    eps_d = din("eps", [1, L_LAYERS], f32)
    al_d = din("alpha", [1, 1], f32)
    ones_d = din("ones128", [1, 128], f32)
    id_d = din("ident", [128, 128], bf16)

    out_d = nc.dram_tensor("out", [NUM_GRAPHS, H], f32, kind="ExternalOutput").ap()

    def _kernel_body(tc):
        with tc.tile_pool(name="persist", bufs=1) as pp:
            s_sb = pp.tile([128, ec], fp8, tag="s")
            d_sb = pp.tile([128, ec], fp8, tag="d")
            b_sb = pp.tile([8, ec], fp8, tag="b")
            hT = pp.tile([128, SK_LOC], bf16, tag="hT")
            h_nm = pp.tile([128, SK_LOC], bf16, tag="hnm")
            p1_sb = pp.tile([128, NT * SG_T], bf16, tag="p1")
            g_sb = pp.tile([128, NQ * NUM_GRAPHS], bf16, tag="g")
            atom_sb = pp.tile([128, H], bf16, tag="atom")
            diff_sb = pp.tile([1, H], bf16, tag="diff")
            rm_sb = pp.tile([1, X_CHUNK], bf16, tag="rm")
            bond_sb = pp.tile([8, H], bf16, tag="bond")
            wp_sb = pp.tile([128, 2 * L_LAYERS * H], bf16, tag="wp")
            bp_sb = pp.tile([128, 2 * L_LAYERS], f32, tag="bp")
            eps_sb = pp.tile([1, L_LAYERS], f32, tag="eps")
            e1bc = pp.tile([128, L_LAYERS], f32, tag="e1bc")
            al_sb = pp.tile([1, 1], f32, tag="al")
            ones_sb = pp.tile([1, 128], f32, tag="ones")
            id_sb = pp.tile([128, 128], bf16, tag="id")
            w_bc = pp.tile([128, S_LOC], f32, tag="wbc")
            rbc = pp.tile([128, S_LOC // M_SUB], f32, tag="rbc")
            ndT = pp.tile([128, NCAN_LOC], f32, tag="ndT")

            # Bulk loads through the idle GPSIMD SWDGE queue; critical-path
            # (embed) loads through SP HWDGE. Chunked so consumers can start
            # before the full tensor lands; layer-critical tensors first.
            nc.gpsimd.dma_start(out=bond_sb, in_=bond_d)
            nc.gpsimd.dma_start(out=wp_sb, in_=wp_d)
            nc.gpsimd.dma_start(out=bp_sb, in_=bp_d)
            nc.gpsimd.dma_start(out=eps_sb, in_=eps_d)
            nc.gpsimd.dma_start(out=ones_sb, in_=ones_d)
            nc.gpsimd.dma_start(out=id_sb, in_=id_d)
            sch = ec // S_CHUNKS
            for i in range(S_CHUNKS):
                sl = slice(i * sch, (i + 1) * sch)
                nc.gpsimd.dma_start(out=s_sb[:, sl], in_=s_d[:, sl])
                nc.gpsimd.dma_start(out=d_sb[:, sl], in_=d_d[:, sl])
                nc.gpsimd.dma_start(out=b_sb[:, sl], in_=b_d[:, sl])
            nc.gpsimd.dma_start(out=p1_sb, in_=p1_d)
            nc.gpsimd.dma_start(out=g_sb, in_=g_d)
            nc.gpsimd.dma_start(out=al_sb, in_=al_d)
            nc.sync.dma_start(out=atom_sb, in_=atom_d)
            nc.sync.dma_start(out=diff_sb, in_=diff_d)
            nc.sync.dma_start(out=rm_sb, in_=rm_d)

            # ---------------- embed ----------------
            with (
                tc.tile_pool(name="emb_sb", bufs=3) as ep,
                tc.tile_pool(name="sm_sb", bufs=1) as smp,
                tc.tile_pool(name="emb_ps", bufs=3, space="PSUM") as epp,
                tc.tile_pool(name="emb_ps1", bufs=1, space="PSUM") as epp1,
                tc.tile_pool(name="emb_ptr", bufs=2, space="PSUM") as eptr,
            ):
                pse = epp1.tile([128, L_LAYERS], f32, tag="pse")
                nc.tensor.matmul(pse, lhsT=ones_sb, rhs=eps_sb, start=True, stop=True)
                nc.scalar.activation(e1bc, pse, AF.Copy, bias=1.0)
                # HT-softmax pooling weights: computed up front while the
                # vector engines are otherwise idle; consumed at pooling.
                rc_sb = smp.tile([1, S_LOC], f32, tag="rc")
                lp_sb = smp.tile([1, S_LOC], f32, tag="lp")
                nc.gpsimd.dma_start(out=rc_sb, in_=rc_d)
                nc.gpsimd.dma_start(out=lp_sb, in_=lp_d)
                nc.vector.tensor_scalar(
                    lp_sb, lp_sb, al_sb[:, 0:1], -1.0, op0=ALU.mult, op1=ALU.mult
                )
                nc.scalar.activation(lp_sb, lp_sb, AF.Exp)
                et = lp_sb
                s4 = smp.tile([1, S_LOC // M_SUB], f32, tag="s4")
                nc.vector.tensor_reduce(
                    s4, et.rearrange("p (a b) -> p a b", b=M_SUB), AX.X, ALU.add
                )
                r4 = smp.tile([1, S_LOC // M_SUB], f32, tag="r4")
                nc.vector.reciprocal(r4, s4)
                nc.vector.tensor_tensor(et, et, rc_sb, ALU.mult)
                wr = et
                for q in range(S_LOC // 512):
                    pw = epp1.tile([128, 512], f32, tag="pw")
                    nc.tensor.matmul(
                        pw, lhsT=ones_sb, rhs=wr[:, q * 512 : (q + 1) * 512],
                        start=True, stop=True,
                    )
                    nc.vector.tensor_copy(w_bc[:, q * 512 : (q + 1) * 512], pw)
                pw = epp1.tile([128, 512], f32, tag="pw")
                nc.tensor.matmul(pw, lhsT=ones_sb, rhs=r4, start=True, stop=True)
                nc.vector.tensor_copy(rbc, pw[:, : S_LOC // M_SUB])
                for q in range(SK_LOC // X_CHUNK):
                    qsl = slice(q * X_CHUNK, (q + 1) * X_CHUNK)
                    xt = ep.tile([128, X_CHUNK], fp8, tag="x")
                    nc.sync.dma_start(out=xt, in_=x_d[:, qsl])
                    ps = epp.tile([128, X_CHUNK], f32, tag="ps")
                    nc.tensor.matmul(ps, lhsT=atom_sb, rhs=xt, start=True, stop=False)
                    nc.tensor.matmul(ps, lhsT=diff_sb, rhs=rm_sb, start=False, stop=True)
                    nc.scalar.activation(hT[:, qsl], ps, AF.Copy)
                    n_pe = EMBED_TRANS.count("P")
                    for k in range(4):
                        if EMBED_TRANS[k] == "P":
                            continue
                        t = q * 4 + k
                        eng = nc.sync if EMBED_TRANS[k] == "S" else nc.scalar
                        eng.dma_start_transpose(
                            h_nm[:, t * 128 : (t + 1) * 128],
                            hT[:, t * 128 : (t + 1) * 128],
                        )
                    if n_pe:
                        assert EMBED_TRANS[:n_pe] == "P" * n_pe
                        ptr = eptr.tile([128, n_pe * 128], bf16, tag="ptr")
                        for k in range(n_pe):
                            t = q * 4 + k
                            nc.tensor.transpose(
                                ptr[:, k * 128 : (k + 1) * 128],
                                hT[:, t * 128 : (t + 1) * 128],
                                id_sb,
                            )
                        nc.vector.tensor_copy(
                            h_nm[:, q * X_CHUNK : q * X_CHUNK + n_pe * 128], ptr
                        )

            # ---------------- layers ----------------
            with (
                tc.tile_pool(name="msg_sb", bufs=3) as mp,
                tc.tile_pool(name="zy_sb", bufs=3) as zp,
                tc.tile_pool(name="ps_m", bufs=PSUM_M, space="PSUM") as pm,
                tc.tile_pool(name="ps_z", bufs=PSUM_Z, space="PSUM") as pz,
                tc.tile_pool(name="ps_mlp", bufs=PSUM_Y, space="PSUM") as pmlp,
                tc.tile_pool(name="ps_tr", bufs=PSUM_T, space="PSUM") as ptp,
            ):
                for l in range(L_LAYERS):
                    w1_l = wp_sb[:, l * H : (l + 1) * H]
                    w2_l = wp_sb[:, (L_LAYERS + l) * H : (L_LAYERS + l + 1) * H]
                    b1_l = bp_sb[:, l : l + 1]
                    b2_l = bp_sb[:, L_LAYERS + l : L_LAYERS + l + 1]
                    for g in range(NG):
                        gsl = slice(g * 512, (g + 1) * 512)
                        psz = pz.tile([128, 512], f32, tag="z")
                        # both pairs' gathers first, so the scatters never
                        # stall the PE on the relu latency
                        psms, msgs = [], []
                        for p in range(2):
                            psm = pm.tile([128, 512], f32, tag="m")
                            psms.append(psm)
                            for tt in range(2):
                                t = g * 4 + p * 2 + tt
                                for ch in range(2):
                                    c0 = t * E_CAP + ch * 128
                                    osl = slice((2 * tt + ch) * 128, (2 * tt + ch + 1) * 128)
                                    nc.tensor.matmul(
                                        psm[:, osl],
                                        lhsT=s_sb[:, c0 : c0 + 128],
                                        rhs=h_nm[:, t * 128 : (t + 1) * 128],
                                        start=True,
                                        stop=False,
                                    )
                                    nc.tensor.matmul(
                                        psm[:, osl],
                                        lhsT=b_sb[:, c0 : c0 + 128],
                                        rhs=bond_sb,
                                        start=False,
                                        stop=True,
                                    )
                            msg = mp.tile([128, 512], bf16, tag="msg")
                            msgs.append(msg)
                            eng = RELU_PAT[(g * 2 + p) % len(RELU_PAT)]
                            if eng == "A":
                                nc.scalar.activation(msg, psm, AF.Relu)
                            else:
                                nc.vector.tensor_scalar_max(msg, psm, 0.0)
                        for p in range(2):
                            msg = msgs[p]
                            for tt in range(2):
                                t = g * 4 + p * 2 + tt
                                tl = p * 2 + tt
                                for ch in range(2):
                                    dc0 = t * E_CAP + ch * 128
                                    nc.tensor.matmul(
                                        psz[:, tl * 128 : (tl + 1) * 128],
                                        lhsT=msg[:, (2 * tt + ch) * 128 : (2 * tt + ch + 1) * 128],
                                        rhs=d_sb[:, dc0 : dc0 + 128],
                                        start=(ch == 0),
                                        stop=(ch == 1),
                                    )
                        zin = zp.tile([128, 512], bf16, tag="zin")
                        nc.vector.scalar_tensor_tensor(
                            zin, hT[:, gsl], e1bc[:, l : l + 1], psz,
                            op0=ALU.mult, op1=ALU.add,
                        )
                        psy = pmlp.tile([128, 512], f32, tag="y")
                        nc.tensor.matmul(psy, lhsT=w1_l, rhs=zin, start=True, stop=True)
                        y1 = zp.tile([128, 512], bf16, tag="y1")
                        if Y1_PAT[g % len(Y1_PAT)] == "A":
                            nc.scalar.activation(y1, psy, AF.Relu, bias=b1_l)
                        else:
                            nc.vector.tensor_scalar(
                                y1, psy, b1_l, 0.0, op0=ALU.add, op1=ALU.max
                            )
                        psz2 = pmlp.tile([128, 512], f32, tag="y")
                        nc.tensor.matmul(psz2, lhsT=w2_l, rhs=y1, start=True, stop=True)
                        oeng = OUT_PAT[g % len(OUT_PAT)]
                        if oeng == "A":
                            nc.scalar.activation(hT[:, gsl], psz2, AF.Identity, bias=b2_l)
                        else:
                            nc.vector.tensor_scalar(
                                hT[:, gsl], psz2, b2_l, None, op0=ALU.add
                            )
                        # hT -> h_nm transposes for next layer / pooling
                        spec = TRANS_SPEC[l]
                        n_pe = spec.count("P")
                        for k in range(4):
                            if spec[k] == "P":
                                continue
                            t = g * 4 + k
                            eng = nc.sync if spec[k] == "S" else nc.scalar
                            eng.dma_start_transpose(
                                h_nm[:, t * 128 : (t + 1) * 128],
                                hT[:, t * 128 : (t + 1) * 128],
                            )
                        if n_pe:
                            assert spec[:n_pe] == "P" * n_pe, "P tiles must lead"
                            ptr = ptp.tile([128, n_pe * 128], bf16, tag="tr")
                            for k in range(n_pe):
                                t = g * 4 + k
                                nc.tensor.transpose(
                                    ptr[:, k * 128 : (k + 1) * 128],
                                    hT[:, t * 128 : (t + 1) * 128],
                                    id_sb,
                                )
                            ceng = COPY_PAT[g % len(COPY_PAT)]
                            csl = slice(g * 512, g * 512 + n_pe * 128)
                            if ceng == "A":
                                nc.scalar.activation(h_nm[:, csl], ptr, AF.Copy)
                            else:
                                nc.vector.tensor_copy(h_nm[:, csl], ptr)
                        if POOL_OVL and l == L_LAYERS - 1 and g % 8 == 7:
                            # subgraph add-pool for the last 8 groups' tiles,
                            # overlapped with the rest of layer 4
                            w0 = g - 7
                            psq = pz.tile([128, 512], f32, tag="z")
                            for ti in range(32):
                                t = w0 * 4 + ti
                                nc.tensor.matmul(
                                    psq[:, ti * SG_T : (ti + 1) * SG_T],
                                    lhsT=h_nm[:, t * 128 : (t + 1) * 128],
                                    rhs=p1_sb[:, t * SG_T : (t + 1) * SG_T],
                                    start=True,
                                    stop=True,
                                )
                            wsl = slice(w0 * 64, w0 * 64 + 512)
                            nc.vector.tensor_tensor(
                                w_bc[:, wsl], psq, w_bc[:, wsl], ALU.mult
                            )
                            nc.vector.tensor_reduce(
                                ndT[:, (g // 8) * 128 : (g // 8 + 1) * 128],
                                w_bc[:, wsl].rearrange("p (a b) -> p a b", b=M_SUB),
                                AX.X,
                                ALU.add,
                            )

            # ---------------- pooling ----------------
            with (
                tc.tile_pool(name="po_sb", bufs=1) as po,
                tc.tile_pool(name="po_big", bufs=1) as pob,
                tc.tile_pool(name="ps_hs", bufs=1, space="PSUM") as phs,
                tc.tile_pool(name="ps_sm1", bufs=1, space="PSUM") as psm_q,
                tc.tile_pool(name="ps_o", bufs=1, space="PSUM") as pso,
            ):
                if not POOL_OVL:
                    hs = phs.tile([128, S_LOC], f32, tag="hs")
                    for t in range(NT):
                        nc.tensor.matmul(
                            hs[:, t * SG_T : (t + 1) * SG_T],
                            lhsT=h_nm[:, t * 128 : (t + 1) * 128],
                            rhs=p1_sb[:, t * SG_T : (t + 1) * SG_T],
                            start=True,
                            stop=True,
                        )
                    wt = w_bc
                    nc.vector.tensor_tensor(wt, hs, w_bc, ALU.mult)
                    nc.vector.tensor_reduce(
                        ndT,
                        wt.rearrange("p (a b) -> p a b", b=M_SUB),
                        AX.X,
                        ALU.add,
                    )
                ndTb = pob.tile([128, NCAN_LOC], bf16, tag="ndTb")
                nc.vector.tensor_tensor(ndTb, ndT, rbc, ALU.mult)
                pout = pso.tile([NUM_GRAPHS, H], f32, tag="po")
                for q in range(NQ):
                    ptq = psm_q.tile([128, 128], bf16, tag="pq")
                    nc.tensor.transpose(ptq, ndTb[:, q * 128 : (q + 1) * 128], id_sb)
                    nnm = po.tile([128, 128], bf16, tag="nnm")
                    nc.vector.tensor_copy(nnm, ptq)
                    nc.tensor.matmul(
                        pout,
                        lhsT=g_sb[:, q * NUM_GRAPHS : (q + 1) * NUM_GRAPHS],
                        rhs=nnm,
                        start=(q == 0),
                        stop=(q == NQ - 1),
                    )
                outs = po.tile([NUM_GRAPHS, H], f32, tag="outs")
                nc.scalar.activation(outs, pout, AF.Copy)
                nc.sync.dma_start(out=out_d, in_=outs)

    with TileContext(nc) as tc:
        if repeat > 1:
            with tc.For_i(0, repeat, 1) as _i:
                _kernel_body(tc)
        else:
            _kernel_body(tc)

    nc.finalize()
    return nc


_CACHE = {}


def _get_bass():
    if "nc" not in _CACHE:
        _CACHE["nc"] = _build_bass()
    return _CACHE["nc"]


def kernel(**inputs):
    from concourse.bass_utils import run_bass_kernel_spmd

    per_core, shared = _host_preprocess(inputs)
    in_maps = [{**pc, **shared} for pc in per_core]
    nc = _get_bass()
    res = run_bass_kernel_spmd(nc, in_maps, core_ids=list(range(NCORES)))
    out = np.zeros((NUM_GRAPHS, H), dtype=np.float32)
    for r in res.results:
        out += np.asarray(r["out"], dtype=np.float32)
    return out
